# revision 2
# baseline (speedup 1.0000x reference)
# Trainium2 Bass kernel for nn_BayesianExpectationTransformerLayer.
#
# Math: attention with no positional encoding / masking is permutation-
# equivariant: _attention(x[:, perm, :]) == _attention(x)[:, perm, :].
# Hence each permuted pass, after applying the inverse permutation, equals
# the standard attention output exactly, and the whole module collapses to
#     out = c * (attention(x) @ Wo^T + bo),
#     c   = (1 - w) + w * variance_reduction_weight,
#     w   = clip(length_adaptive_weight * log(S)/S, 0.01, 1.0)
#
# Device strategy (8 NeuronCores, SPMD, tensor-parallel over heads):
#   - core c owns heads 2c, 2c+1 (feature slice F = 128 of D = 1024)
#   - per core: QT/KT = [F, S] per-batch projections (f32r matmuls), V via
#     a transposed projection + PE transpose, S^T = K Q^T scores per
#     (batch, head), exp to bf16 (softmax without max-subtraction:
#     |scores| < 7), AV in bf16 with an appended ones-column producing the
#     softmax denominator, per-partition 1/denom scale fused into the
#     PSUM->SBUF read, PE transpose (rhs=identity) to A^T, two half
#     AllToAlls re-shard head-split -> row-split, out-projection emits the
#     final [256, 1024] rows.
#   - throughput: the rep loop is software-pipelined -- rep r's
#     out-projection is emitted after rep r+1's first projections and the
#     A2A DRAM tiles are double-buffered, so the collectives overlap the
#     next rep's compute; PE emission interleaves scores with projections
#     so the in-order PE never stalls on the Activation engine's exp.
#   - host: folds scale/c into the weights, builds x^T, concatenates the
#     per-core row slices.

import os
import sys

for _p in ("/opt/trn_rl_repo", "/root/.axon_site/_ro/trn_rl_repo"):
    if os.path.isdir(_p) and _p not in sys.path:
        sys.path.append(_p)

import numpy as np

import concourse.bass as bass
import concourse.mybir as mybir
import concourse.tile as tile
from concourse import bacc
from concourse.bass import ts
from concourse.bass_utils import run_bass_kernel_spmd
from concourse.masks import make_identity

B, S, D = 2, 1024, 1024
H, HD = 16, 64
KPERM = 20
NCORES = 8
HPC = H // NCORES          # heads per core = 2
F = HPC * HD               # per-core feature slice = 128
R = B * S                  # 2048 rows
RPC = R // NCORES          # output rows per core = 256
FP32 = mybir.dt.float32
BF16 = mybir.dt.bfloat16
F32R = mybir.dt.float32r

TRACE = False
LAST = None

_CACHED = {}

NKC = S // 128             # 8 k-chunks per sequence
NQC2 = S // 512            # 2 q-chunks of 512 per sequence


def _build(reps=1):
    nc = bacc.Bacc(None)

    xT = nc.declare_dram_parameter("xT", [D, R], BF16, isOutput=False)
    wqT = nc.declare_dram_parameter("wqT", [D, F], BF16, isOutput=False)
    wkT = nc.declare_dram_parameter("wkT", [D, F], BF16, isOutput=False)
    wvT = nc.declare_dram_parameter("wvT", [D, F], BF16, isOutput=False)
    woT = nc.declare_dram_parameter("woT", [D, D], BF16, isOutput=False)
    bqs = nc.declare_dram_parameter("bqs", [F, 1], FP32, isOutput=False)
    bks = nc.declare_dram_parameter("bks", [F, 1], FP32, isOutput=False)
    bvb = nc.declare_dram_parameter("bvb", [128, HPC, HD], BF16, isOutput=False)
    out = nc.declare_dram_parameter("out", [RPC, D], FP32, isOutput=True)

    Exp = mybir.ActivationFunctionType.Exp
    Copy = mybir.ActivationFunctionType.Copy

    with tile.TileContext(nc) as tc:
        with (
            tc.tile_pool(name="const", bufs=1) as cpool,
            tc.tile_pool(name="xt", bufs=2) as xtpool,
            tc.tile_pool(name="qkv", bufs=2) as qkvpool,
            tc.tile_pool(name="vnat", bufs=2) as vpool,
            tc.tile_pool(name="pt", bufs=2) as ptpool,
            tc.tile_pool(name="sm", bufs=12) as smpool,
            tc.tile_pool(name="osb", bufs=4) as opool,
            tc.tile_pool(name="ps_big", bufs=2, space="PSUM") as psb,
            tc.tile_pool(name="ps_small", bufs=4, space="PSUM") as pss,
            tc.tile_pool(name="dram", bufs=2, space="DRAM") as dpool,
        ):
            # ---- constants ----
            ident = cpool.tile([128, 128], FP32, tag="ident")
            make_identity(nc, ident[:])
            ident_bf = cpool.tile([128, 128], BF16, tag="ident_bf")
            nc.vector.tensor_copy(ident_bf[:], ident[:])

            wq_sb = cpool.tile([128, 8, F], BF16, tag="wq")
            wk_sb = cpool.tile([128, 8, F], BF16, tag="wk")
            wv_sb = cpool.tile([128, 8, F], BF16, tag="wv")
            nc.sync.dma_start(wq_sb[:], wqT[:].rearrange("(c p) f -> p c f", p=128))
            nc.sync.dma_start(wk_sb[:], wkT[:].rearrange("(c p) f -> p c f", p=128))
            nc.gpsimd.dma_start(wv_sb[:], wvT[:].rearrange("(c p) f -> p c f", p=128))
            bq_sb = cpool.tile([F, 1], FP32, tag="bq")
            bk_sb = cpool.tile([F, 1], FP32, tag="bk")
            bv_sb = cpool.tile([128, HPC, HD], BF16, tag="bv")
            nc.sync.dma_start(bq_sb[:], bqs[:])
            nc.sync.dma_start(bk_sb[:], bks[:])
            nc.sync.dma_start(bv_sb[:], bvb[:])
            wof = cpool.tile([128, 8, D], BF16, tag="wof")
            nc.gpsimd.dma_start(wof[:], woT[:].rearrange("(c p) d -> p c d", p=128))

            xTr = xT[:].rearrange("(c p) r -> p c r", p=128)

            def emit_proj(rep, b):
                """Projections for batch b: returns (QT, KT, V0, V1) tiles and
                a list of deferred PE thunks (to interleave with scores)."""
                QT = qkvpool.tile([128, S], F32R, tag="qt", name=f"qt_{rep}_{b}")
                KT = qkvpool.tile([128, S], F32R, tag="kt", name=f"kt_{rep}_{b}")
                VT = qkvpool.tile([128, S], BF16, tag="vt", name=f"vt_{rep}_{b}")
                V0 = vpool.tile([128, NKC, HD + 1], BF16, tag="v0",
                                name=f"v0_{rep}_{b}")
                V1 = vpool.tile([128, NKC, HD + 1], BF16, tag="v1",
                                name=f"v1_{rep}_{b}")
                nc.vector.memset(V0[:, :, HD : HD + 1], 1.0)
                nc.vector.memset(V1[:, :, HD : HD + 1], 1.0)

                thunks = []
                def load_xt():
                    xt = xtpool.tile([128, 8, S], BF16, tag="xt",
                                     name=f"xt_{rep}_{b}")
                    xb = xTr[:, :, ts(b, S)]
                    nc.sync.dma_start(xt[:, 0:4, :], xb[:, 0:4, :])
                    nc.gpsimd.dma_start(xt[:, 4:8, :], xb[:, 4:8, :])
                    return xt

                xt_holder = []

                def get_xt(h=xt_holder):
                    if not h:
                        h.append(load_xt())
                    return h[0]

                # wide 1024-col accumulation chains: the matmul moving dim
                # caps at 512, so each dc feeds two 512 matmuls into one
                # [128, 1024] PSUM tile; bias/copy drains it in one DVE op.
                for w_sb, b_sb, dst in (
                    (wq_sb, bq_sb, QT), (wk_sb, bk_sb, KT), (wv_sb, None, VT)
                ):
                    def proj_wide(w_sb=w_sb, b_sb=b_sb, dst=dst):
                        xt = get_xt()
                        ps = psb.tile([128, S], FP32, tag="mm512",
                                      name=f"ps_{rep}_{b}_{id(w_sb)}")
                        for dc in range(8):
                            for q2 in range(2):
                                nc.tensor.matmul(
                                    ps[:, ts(q2, 512)],
                                    lhsT=w_sb[:, dc, :],
                                    rhs=xt[:, dc, ts(q2, 512)],
                                    start=(dc == 0), stop=(dc == 7),
                                )
                        if b_sb is not None:
                            nc.vector.tensor_scalar_add(
                                dst[:], ps[:], b_sb[:, 0:1]
                            )
                        else:
                            nc.vector.tensor_copy(dst[:], ps[:])
                    thunks.append(proj_wide)

                for c8 in range(8):
                    def vtr(c8=c8):
                        tp = pss.tile([128, 128], BF16, tag="mm128",
                                      name=f"tp_{rep}_{b}_{c8}")
                        nc.tensor.transpose(tp[:], VT[:, ts(c8, 128)],
                                            ident_bf[:])
                        for h, Vh in ((0, V0), (1, V1)):
                            nc.vector.tensor_add(
                                Vh[:, c8, 0:HD], tp[:, ts(h, HD)], bv_sb[:, h, :]
                            )
                    thunks.append(vtr)
                return QT, KT, V0, V1, thunks

            def emit_scores(rep, b, QT, KT):
                """Score+exp thunk list: 16 chunks (h-major), each one PE
                st-matmul pair + one Act exp."""
                pt0 = ptpool.tile([128, NKC, S], BF16, tag="pt0",
                                  name=f"pt0_{rep}_{b}")
                pt1 = ptpool.tile([128, NKC, S], BF16, tag="pt1",
                                  name=f"pt1_{rep}_{b}")
                pts = (pt0, pt1)
                thunks = []
                for h in range(HPC):
                    for kc in range(NKC):
                        def sc(h=h, kc=kc):
                            st = psb.tile([128, S], FP32, tag="mm512",
                                          name=f"st_{rep}_{b}_{h}_{kc}")
                            for qc2 in range(NQC2):
                                nc.tensor.matmul(
                                    st[:, ts(qc2, 512)],
                                    lhsT=KT[ts(h, HD), ts(kc, 128)],
                                    rhs=QT[ts(h, HD), ts(qc2, 512)],
                                    start=True, stop=True,
                                )
                            nc.scalar.activation(pts[h][:, kc, :], st[:], Exp)
                        thunks.append(sc)
                return pt0, pt1, thunks

            def emit_av_qc(rep, b, qc, pt0, pt1, V0, V1, a2a_halves):
                """AV + normalize + transpose + stage for one q-chunk."""
                at_ps = psb.tile([128, 128], FP32, tag="mm512",
                                 name=f"at_{rep}_{b}_{qc}")
                for h, pth, Vh in ((0, pt0, V0), (1, pt1, V1)):
                    av = pss.tile([128, HD + 1], FP32, tag="mm128",
                                  name=f"av_{rep}_{b}_{qc}_{h}")
                    for kc in range(NKC):
                        nc.tensor.matmul(
                            av[:],
                            lhsT=pth[:, kc, ts(qc, 128)],
                            rhs=Vh[:, kc, :],
                            start=(kc == 0), stop=(kc == 7),
                        )
                    recip = smpool.tile([128, 1], FP32, tag="recip")
                    nc.vector.reciprocal(recip[:], av[:, HD : HD + 1])
                    asb = smpool.tile([128, HD], FP32, tag="asb")
                    nc.vector.tensor_scalar_mul(asb[:], av[:, 0:HD], recip[:, 0:1])
                    nc.tensor.matmul(
                        at_ps[ts(h, HD), :], lhsT=asb[:], rhs=ident[:],
                        start=True, stop=True, tile_position=(0, h * HD),
                    )
                at_sb = smpool.tile([128, 128], BF16, tag="at_sb")
                if qc % 2 == 0:
                    nc.scalar.activation(at_sb[:], at_ps[:], Copy)
                else:
                    nc.vector.tensor_copy(at_sb[:], at_ps[:])
                j = b * 4 + qc // 2
                nc.sync.dma_start(a2a_halves[qc % 2][j], at_sb[:])

            def emit_outproj(a2a_out0, a2a_out1, rep_tag):
                """Called one rep after the A2A it consumes (its waits are
                satisfied, so no in-order queue blocks behind it).  Emits the
                atf load DMAs immediately; returns the PE/store thunks for
                proportional mixing into the caller's phase."""
                thunks = []
                for rsub, a2a_o in ((0, a2a_out0), (1, a2a_out1)):
                    atf = opool.tile([128, 8, 128], BF16, tag=f"atf{rsub}",
                                     name=f"atf{rsub}_{rep_tag}")
                    a2a_or = a2a_o.rearrange("c p r -> p c r")
                    nc.sync.dma_start(atf[:, 0:4, :], a2a_or[:, 0:4, :])
                    nc.gpsimd.dma_start(atf[:, 4:8, :], a2a_or[:, 4:8, :])
                    for dc in range(2):
                        def po_one(rsub=rsub, dc=dc, atf=atf):
                            po = psb.tile([128, 512], FP32, tag="mm512",
                                          name=f"po_{rep_tag}_{rsub}_{dc}")
                            for fc in range(8):
                                nc.tensor.matmul(
                                    po[:],
                                    lhsT=atf[:, fc, :],
                                    rhs=wof[:, fc, ts(dc, 512)],
                                    start=(fc == 0), stop=(fc == 7),
                                )
                            o_sb = opool.tile([128, 512], FP32, tag="osb")
                            nc.vector.tensor_copy(o_sb[:], po[:])
                            odma = nc.sync if dc == 0 else nc.gpsimd
                            odma.dma_start(out[ts(rsub, 128), ts(dc, 512)], o_sb[:])
                        thunks.append(po_one)
                return thunks

            def run_mixed(*streams):
                """Emit thunks from several streams, proportionally mixed."""
                streams = [list(s) for s in streams if s]
                idx = [0] * len(streams)
                while True:
                    best, bestfrac = -1, 2.0
                    for i, s in enumerate(streams):
                        if idx[i] < len(s):
                            frac = idx[i] / len(s)
                            if frac < bestfrac:
                                best, bestfrac = i, frac
                    if best < 0:
                        break
                    streams[best][idx[best]]()
                    idx[best] += 1

            prev_a2a = None

            for rep in range(reps):
                a2a_in = dpool.tile([NCORES, 2, 128, 128], BF16, tag="a2a_in",
                                    name=f"a2a_in_{rep}")
                a2a_out = dpool.tile([NCORES, 2, 128, 128], BF16, tag="a2a_out",
                                     name=f"a2a_out_{rep}")
                a2a_halves = (a2a_in[:, 0], a2a_in[:, 1])

                QT0, KT0, V00, V01, proj0 = emit_proj(rep, 0)
                run_mixed(proj0)

                # batch 0 scores interleaved with batch 1 projections: Act
                # grinds exp while PE does b1's GEMMs.
                pt00, pt01, sc0 = emit_scores(rep, 0, QT0, KT0)
                QT1, KT1, V10, V11, proj1 = emit_proj(rep, 1)
                run_mixed(sc0, proj1)

                # batch 0 AV interleaved with batch 1 scores and the
                # previous rep's out-projection (whose A2A finished at the
                # end of rep-1, so nothing here ever waits).
                sc1_pt = emit_scores(rep, 1, QT1, KT1)
                pt10, pt11, sc1 = sc1_pt
                av0 = [
                    (lambda qc=qc: emit_av_qc(rep, 0, qc, pt00, pt01, V00, V01,
                                              a2a_halves))
                    for qc in range(8)
                ]
                po_prev = (emit_outproj(prev_a2a[:, 0], prev_a2a[:, 1], rep - 1)
                           if prev_a2a is not None else [])
                run_mixed(av0, sc1, po_prev)

                # batch 1 AV: evens first, then A2A0 (its payload is
                # complete), then odds, then A2A1.
                for qc in range(8):
                    emit_av_qc(rep, 1, qc, pt10, pt11, V10, V11, a2a_halves)
                nc.gpsimd.collective_compute(
                    "AllToAll", mybir.AluOpType.bypass,
                    replica_groups=[list(range(NCORES))],
                    ins=[a2a_in.opt()], outs=[a2a_out.opt()],
                )

                prev_a2a = a2a_out

            for t in emit_outproj(prev_a2a[:, 0], prev_a2a[:, 1], reps - 1):
                t()

    nc.finalize()
    return nc


def _get_nc(reps=1):
    global _CACHED
    if _CACHED is None:
        _CACHED = {}
    if reps not in _CACHED:
        _CACHED[reps] = _build(reps)
    return _CACHED[reps]


def _make_in_maps(x2d, Wq, bq, Wk, bk, Wv, bv, woT_eff):
    import ml_dtypes
    bf16 = ml_dtypes.bfloat16
    sm_scale = np.float32(1.0 / np.sqrt(HD))
    xT_full = np.ascontiguousarray(x2d.T).astype(bf16)
    woT_eff = np.ascontiguousarray(woT_eff).astype(bf16)

    in_maps = []
    for c in range(NCORES):
        hs = slice(c * F, (c + 1) * F)
        in_maps.append({
            "xT": xT_full,
            "wqT": np.ascontiguousarray((sm_scale * Wq[hs, :]).T).astype(bf16),
            "wkT": np.ascontiguousarray(Wk[hs, :].T).astype(bf16),
            "wvT": np.ascontiguousarray(Wv[hs, :].T).astype(bf16),
            "woT": woT_eff,
            "bqs": np.ascontiguousarray((sm_scale * bq[hs])[:, None]),
            "bks": np.ascontiguousarray(bk[hs][:, None]),
            "bvb": np.ascontiguousarray(
                np.broadcast_to(bv[hs].reshape(HPC, HD)[None], (128, HPC, HD))
            ).astype(bf16),
        })
    return in_maps


def _run_pass(x2d, Wq, bq, Wk, bk, Wv, bv, woT_eff):
    global LAST
    nc = _get_nc()
    in_maps = _make_in_maps(x2d, Wq, bq, Wk, bk, Wv, bv, woT_eff)
    res = run_bass_kernel_spmd(nc, in_maps, list(range(NCORES)), trace=TRACE)
    LAST = res
    return np.concatenate([res.results[c]["out"] for c in range(NCORES)], axis=0)


def kernel(x, Wq, bq, Wk, bk, Wv, bv, Wo, bo,
           variance_reduction_weight, length_adaptive_weight, perms):
    x = np.asarray(x, dtype=np.float32)
    Wq, bq = np.asarray(Wq, np.float32), np.asarray(bq, np.float32)
    Wk, bk = np.asarray(Wk, np.float32), np.asarray(bk, np.float32)
    Wv, bv = np.asarray(Wv, np.float32), np.asarray(bv, np.float32)
    Wo, bo = np.asarray(Wo, np.float32), np.asarray(bo, np.float32)
    perms = np.asarray(perms)
    b, s, d = x.shape

    law = float(np.asarray(length_adaptive_weight).reshape(-1)[0])
    vrw = float(np.asarray(variance_reduction_weight).reshape(-1)[0])
    w = np.float32(min(max(law * np.log(s) / s, 0.01), 1.0))
    x2d = x.reshape(R, D)

    is_perm = all(
        np.array_equal(np.sort(np.asarray(perms[i])), np.arange(s))
        for i in range(perms.shape[0])
    )

    if is_perm:
        c = (1.0 - w) + w * vrw
        outp = _run_pass(x2d, Wq, bq, Wk, bk, Wv, bv, (c * Wo).T)
        outp = outp + (c * bo)[None, :]
        return outp.reshape(b, s, d).astype(np.float32)

    acc = _run_pass(x2d, Wq, bq, Wk, bk, Wv, bv, ((1.0 - w) * Wo).T)
    pscale = (w * vrw) / np.float32(perms.shape[0])
    for i in range(perms.shape[0]):
        perm = np.asarray(perms[i]).astype(np.int64)
        xp = x[:, perm, :].reshape(R, D)
        op = _run_pass(xp, Wq, bq, Wk, bk, Wv, bv, (pscale * Wo).T)
        op3 = op.reshape(b, s, d)
        inv = np.argsort(perm)
        acc += op3[:, inv, :].reshape(R, D)
    acc = acc + (((1.0 - w) + w * vrw) * bo)[None, :]
    return acc.reshape(b, s, d).astype(np.float32)


# revision 3
# speedup vs baseline: 1.3746x; 1.3746x over previous
# Trainium2 Bass kernel for nn_BayesianExpectationTransformerLayer.
#
# Math: attention with no positional encoding / masking is permutation-
# equivariant: _attention(x[:, perm, :]) == _attention(x)[:, perm, :].
# Hence each permuted pass, after applying the inverse permutation, equals
# the standard attention output exactly, and the whole module collapses to
#     out = c * (attention(x) @ Wo^T + bo),
#     c   = (1 - w) + w * variance_reduction_weight,
#     w   = clip(length_adaptive_weight * log(S)/S, 0.01, 1.0)
#
# Device strategy (8 NeuronCores, SPMD, tensor-parallel over heads):
#   - core c owns heads 2c, 2c+1 (feature slice F = 128 of D = 1024)
#   - per core: QT/KT = [F, S] per-batch projections (f32r matmuls), V via
#     a transposed projection + PE transpose, S^T = K Q^T scores per
#     (batch, head), exp to bf16 (softmax without max-subtraction:
#     |scores| < 7), AV in bf16 with an appended ones-column producing the
#     softmax denominator, per-partition 1/denom scale fused into the
#     PSUM->SBUF read, PE transpose (rhs=identity) to A^T, two half
#     AllToAlls re-shard head-split -> row-split, out-projection emits the
#     final [256, 1024] rows.
#   - throughput: the rep loop is software-pipelined -- rep r's
#     out-projection is emitted after rep r+1's first projections and the
#     A2A DRAM tiles are double-buffered, so the collectives overlap the
#     next rep's compute; PE emission interleaves scores with projections
#     so the in-order PE never stalls on the Activation engine's exp.
#   - host: folds scale/c into the weights, builds x^T, concatenates the
#     per-core row slices.

import os
import sys

for _p in ("/opt/trn_rl_repo", "/root/.axon_site/_ro/trn_rl_repo"):
    if os.path.isdir(_p) and _p not in sys.path:
        sys.path.append(_p)

import numpy as np

import concourse.bass as bass
import concourse.mybir as mybir
import concourse.tile as tile
from concourse import bacc
from concourse.bass import ts
from concourse.bass_utils import run_bass_kernel_spmd
from concourse.masks import make_identity

B, S, D = 2, 1024, 1024
H, HD = 16, 64
KPERM = 20
NCORES = 8
HPC = H // NCORES          # heads per core = 2
F = HPC * HD               # per-core feature slice = 128
R = B * S                  # 2048 rows
RPC = R // NCORES          # output rows per core = 256
FP32 = mybir.dt.float32
BF16 = mybir.dt.bfloat16
F32R = mybir.dt.float32r

TRACE = False
LAST = None

_CACHED = {}

NKC = S // 128             # 8 k-chunks per sequence
NQC2 = S // 512            # 2 q-chunks of 512 per sequence


def _build(reps=1):
    nc = bacc.Bacc(None)

    xT = nc.declare_dram_parameter("xT", [D, R], BF16, isOutput=False)
    wqT = nc.declare_dram_parameter("wqT", [D, F], BF16, isOutput=False)
    wkT = nc.declare_dram_parameter("wkT", [D, F], BF16, isOutput=False)
    wvT = nc.declare_dram_parameter("wvT", [D, F], BF16, isOutput=False)
    woT = nc.declare_dram_parameter("woT", [D, D], BF16, isOutput=False)
    bqs = nc.declare_dram_parameter("bqs", [F, 1], FP32, isOutput=False)
    bks = nc.declare_dram_parameter("bks", [F, 1], FP32, isOutput=False)
    bvb = nc.declare_dram_parameter("bvb", [128, HPC, HD], BF16, isOutput=False)
    out = nc.declare_dram_parameter("out", [RPC, D], FP32, isOutput=True)

    Exp = mybir.ActivationFunctionType.Exp
    Copy = mybir.ActivationFunctionType.Copy

    with tile.TileContext(nc) as tc:
        with (
            tc.tile_pool(name="const", bufs=1) as cpool,
            tc.tile_pool(name="xt", bufs=2) as xtpool,
            tc.tile_pool(name="qkv", bufs=2) as qkvpool,
            tc.tile_pool(name="vnat", bufs=2) as vpool,
            tc.tile_pool(name="pt", bufs=2) as ptpool,
            tc.tile_pool(name="sm", bufs=12) as smpool,
            tc.tile_pool(name="osb", bufs=4) as opool,
            tc.tile_pool(name="ps_big", bufs=2, space="PSUM") as psb,
            tc.tile_pool(name="ps_small", bufs=4, space="PSUM") as pss,
            tc.tile_pool(name="dram", bufs=2, space="DRAM") as dpool,
        ):
            # ---- constants ----
            ident = cpool.tile([128, 128], FP32, tag="ident")
            make_identity(nc, ident[:])
            ident_bf = cpool.tile([128, 128], BF16, tag="ident_bf")
            nc.vector.tensor_copy(ident_bf[:], ident[:])

            wq_sb = cpool.tile([128, 8, F], BF16, tag="wq")
            wk_sb = cpool.tile([128, 8, F], BF16, tag="wk")
            wv_sb = cpool.tile([128, 8, F], BF16, tag="wv")
            nc.sync.dma_start(wq_sb[:], wqT[:].rearrange("(c p) f -> p c f", p=128))
            nc.sync.dma_start(wk_sb[:], wkT[:].rearrange("(c p) f -> p c f", p=128))
            nc.gpsimd.dma_start(wv_sb[:], wvT[:].rearrange("(c p) f -> p c f", p=128))
            bq_sb = cpool.tile([F, 1], FP32, tag="bq")
            bk_sb = cpool.tile([F, 1], FP32, tag="bk")
            bv_sb = cpool.tile([128, HPC, HD], BF16, tag="bv")
            nc.sync.dma_start(bq_sb[:], bqs[:])
            nc.sync.dma_start(bk_sb[:], bks[:])
            nc.sync.dma_start(bv_sb[:], bvb[:])
            wof = cpool.tile([128, 8, D], BF16, tag="wof")
            nc.gpsimd.dma_start(wof[:], woT[:].rearrange("(c p) d -> p c d", p=128))

            xTr = xT[:].rearrange("(c p) r -> p c r", p=128)

            def emit_proj(rep, b):
                """Projections for batch b: returns (QT, KT, V0, V1) tiles and
                a list of deferred PE thunks (to interleave with scores)."""
                QT = qkvpool.tile([128, S], F32R, tag="qt", name=f"qt_{rep}_{b}")
                KT = qkvpool.tile([128, S], F32R, tag="kt", name=f"kt_{rep}_{b}")
                VT = qkvpool.tile([128, S], BF16, tag="vt", name=f"vt_{rep}_{b}")
                V0 = vpool.tile([128, NKC, HD + 1], BF16, tag="v0",
                                name=f"v0_{rep}_{b}")
                V1 = vpool.tile([128, NKC, HD + 1], BF16, tag="v1",
                                name=f"v1_{rep}_{b}")
                nc.vector.memset(V0[:, :, HD : HD + 1], 1.0)
                nc.vector.memset(V1[:, :, HD : HD + 1], 1.0)

                thunks = []
                def load_xt():
                    xt = xtpool.tile([128, 8, S], BF16, tag="xt",
                                     name=f"xt_{rep}_{b}")
                    xb = xTr[:, :, ts(b, S)]
                    nc.sync.dma_start(xt[:, 0:4, :], xb[:, 0:4, :])
                    nc.gpsimd.dma_start(xt[:, 4:8, :], xb[:, 4:8, :])
                    return xt

                xt_holder = []

                def get_xt(h=xt_holder):
                    if not h:
                        h.append(load_xt())
                    return h[0]

                # wide 1024-col accumulation chains: the matmul moving dim
                # caps at 512, so each dc feeds two 512 matmuls into one
                # [128, 1024] PSUM tile; bias/copy drains it in one DVE op.
                for w_sb, b_sb, dst in (
                    (wq_sb, bq_sb, QT), (wk_sb, bk_sb, KT), (wv_sb, None, VT)
                ):
                    def proj_wide(w_sb=w_sb, b_sb=b_sb, dst=dst):
                        xt = get_xt()
                        ps = psb.tile([128, S], FP32, tag="mm512",
                                      name=f"ps_{rep}_{b}_{id(w_sb)}")
                        for dc in range(8):
                            for q2 in range(2):
                                nc.tensor.matmul(
                                    ps[:, ts(q2, 512)],
                                    lhsT=w_sb[:, dc, :],
                                    rhs=xt[:, dc, ts(q2, 512)],
                                    start=(dc == 0), stop=(dc == 7),
                                )
                        if b_sb is not None:
                            nc.vector.tensor_scalar_add(
                                dst[:], ps[:], b_sb[:, 0:1]
                            )
                        else:
                            nc.vector.tensor_copy(dst[:], ps[:])
                    thunks.append(proj_wide)

                for c8 in range(8):
                    def vtr(c8=c8):
                        tp = pss.tile([128, 128], BF16, tag="mm128",
                                      name=f"tp_{rep}_{b}_{c8}")
                        nc.tensor.transpose(tp[:], VT[:, ts(c8, 128)],
                                            ident_bf[:])
                        for h, Vh in ((0, V0), (1, V1)):
                            nc.vector.tensor_add(
                                Vh[:, c8, 0:HD], tp[:, ts(h, HD)], bv_sb[:, h, :]
                            )
                    thunks.append(vtr)
                return QT, KT, V0, V1, thunks

            def emit_scores(rep, b, QT, KT):
                """Score+exp thunk list: 16 chunks (h-major), each one PE
                st-matmul pair + one Act exp."""
                pt0 = ptpool.tile([128, NKC, S], BF16, tag="pt0",
                                  name=f"pt0_{rep}_{b}")
                pt1 = ptpool.tile([128, NKC, S], BF16, tag="pt1",
                                  name=f"pt1_{rep}_{b}")
                pts = (pt0, pt1)
                thunks = []
                for h in range(HPC):
                    for kc in range(NKC):
                        def sc(h=h, kc=kc):
                            st = psb.tile([128, S], FP32, tag="mm512",
                                          name=f"st_{rep}_{b}_{h}_{kc}")
                            for qc2 in range(NQC2):
                                nc.tensor.matmul(
                                    st[:, ts(qc2, 512)],
                                    lhsT=KT[ts(h, HD), ts(kc, 128)],
                                    rhs=QT[ts(h, HD), ts(qc2, 512)],
                                    start=True, stop=True,
                                )
                            nc.scalar.activation(pts[h][:, kc, :], st[:], Exp)
                        thunks.append(sc)
                return pt0, pt1, thunks

            def emit_av_qc(rep, b, qc, pt0, pt1, V0, V1, a2a_halves):
                """AV + normalize + transpose + stage for one q-chunk."""
                at_ps = psb.tile([128, 128], FP32, tag="mm512",
                                 name=f"at_{rep}_{b}_{qc}")
                for h, pth, Vh in ((0, pt0, V0), (1, pt1, V1)):
                    av = pss.tile([128, HD + 1], FP32, tag="mm128",
                                  name=f"av_{rep}_{b}_{qc}_{h}")
                    for kc in range(NKC):
                        nc.tensor.matmul(
                            av[:],
                            lhsT=pth[:, kc, ts(qc, 128)],
                            rhs=Vh[:, kc, :],
                            start=(kc == 0), stop=(kc == 7),
                        )
                    recip = smpool.tile([128, 1], FP32, tag="recip")
                    nc.vector.reciprocal(recip[:], av[:, HD : HD + 1])
                    asb = smpool.tile([128, HD], FP32, tag="asb")
                    nc.vector.tensor_scalar_mul(asb[:], av[:, 0:HD], recip[:, 0:1])
                    nc.tensor.matmul(
                        at_ps[ts(h, HD), :], lhsT=asb[:], rhs=ident[:],
                        start=True, stop=True, tile_position=(0, h * HD),
                    )
                at_sb = smpool.tile([128, 128], BF16, tag="at_sb")
                # keep the Activation queue a pure exp pipeline: a copy here
                # pends on PE's at-norm and would stall the next batch's exps
                nc.vector.tensor_copy(at_sb[:], at_ps[:])
                j = b * 4 + qc // 2
                nc.sync.dma_start(a2a_halves[qc % 2][j], at_sb[:])

            def emit_outproj(a2a_out0, a2a_out1, rep_tag):
                """Called one rep after the A2A it consumes (its waits are
                satisfied, so no in-order queue blocks behind it).  Emits the
                atf load DMAs immediately; returns the PE/store thunks for
                proportional mixing into the caller's phase."""
                thunks = []
                for rsub, a2a_o in ((0, a2a_out0), (1, a2a_out1)):
                    atf = opool.tile([128, 8, 128], BF16, tag=f"atf{rsub}",
                                     name=f"atf{rsub}_{rep_tag}")
                    a2a_or = a2a_o.rearrange("c p r -> p c r")
                    nc.sync.dma_start(atf[:, 0:4, :], a2a_or[:, 0:4, :])
                    nc.gpsimd.dma_start(atf[:, 4:8, :], a2a_or[:, 4:8, :])
                    for dc in range(2):
                        def po_one(rsub=rsub, dc=dc, atf=atf):
                            po = psb.tile([128, 512], FP32, tag="mm512",
                                          name=f"po_{rep_tag}_{rsub}_{dc}")
                            for fc in range(8):
                                nc.tensor.matmul(
                                    po[:],
                                    lhsT=atf[:, fc, :],
                                    rhs=wof[:, fc, ts(dc, 512)],
                                    start=(fc == 0), stop=(fc == 7),
                                )
                            o_sb = opool.tile([128, 512], FP32, tag="osb")
                            nc.vector.tensor_copy(o_sb[:], po[:])
                            odma = nc.sync if dc == 0 else nc.gpsimd
                            odma.dma_start(out[ts(rsub, 128), ts(dc, 512)], o_sb[:])
                        thunks.append(po_one)
                return thunks

            def run_mixed(*streams):
                """Emit thunks from several streams, proportionally mixed."""
                streams = [list(s) for s in streams if s]
                idx = [0] * len(streams)
                while True:
                    best, bestfrac = -1, 2.0
                    for i, s in enumerate(streams):
                        if idx[i] < len(s):
                            frac = idx[i] / len(s)
                            if frac < bestfrac:
                                best, bestfrac = i, frac
                    if best < 0:
                        break
                    streams[best][idx[best]]()
                    idx[best] += 1

            prev_a2a = None

            for rep in range(reps):
                a2a_in = dpool.tile([NCORES, 2, 128, 128], BF16, tag="a2a_in",
                                    name=f"a2a_in_{rep}")
                a2a_out = dpool.tile([NCORES, 2, 128, 128], BF16, tag="a2a_out",
                                     name=f"a2a_out_{rep}")
                a2a_halves = (a2a_in[:, 0], a2a_in[:, 1])

                QT0, KT0, V00, V01, proj0 = emit_proj(rep, 0)
                run_mixed(proj0)

                # batch 0 scores interleaved with batch 1 projections: Act
                # grinds exp while PE does b1's GEMMs.
                pt00, pt01, sc0 = emit_scores(rep, 0, QT0, KT0)
                QT1, KT1, V10, V11, proj1 = emit_proj(rep, 1)
                run_mixed(sc0, proj1)

                # batch 0 AV interleaved with batch 1 scores and the
                # previous rep's out-projection (whose A2A finished at the
                # end of rep-1, so nothing here ever waits).
                sc1_pt = emit_scores(rep, 1, QT1, KT1)
                pt10, pt11, sc1 = sc1_pt
                av0 = [
                    (lambda qc=qc: emit_av_qc(rep, 0, qc, pt00, pt01, V00, V01,
                                              a2a_halves))
                    for qc in range(8)
                ]
                po_prev = (emit_outproj(prev_a2a[:, 0], prev_a2a[:, 1], rep - 1)
                           if prev_a2a is not None else [])
                run_mixed(av0, sc1, po_prev)

                # batch 1 AV: evens first, then A2A0 (its payload is
                # complete), then odds, then A2A1.
                for qc in range(8):
                    emit_av_qc(rep, 1, qc, pt10, pt11, V10, V11, a2a_halves)
                nc.gpsimd.collective_compute(
                    "AllToAll", mybir.AluOpType.bypass,
                    replica_groups=[list(range(NCORES))],
                    ins=[a2a_in.opt()], outs=[a2a_out.opt()],
                )

                prev_a2a = a2a_out

            for t in emit_outproj(prev_a2a[:, 0], prev_a2a[:, 1], reps - 1):
                t()

    nc.finalize()
    return nc


def _get_nc(reps=1):
    global _CACHED
    if _CACHED is None:
        _CACHED = {}
    if reps not in _CACHED:
        _CACHED[reps] = _build(reps)
    return _CACHED[reps]


def _make_in_maps(x2d, Wq, bq, Wk, bk, Wv, bv, woT_eff):
    import ml_dtypes
    bf16 = ml_dtypes.bfloat16
    sm_scale = np.float32(1.0 / np.sqrt(HD))
    xT_full = np.ascontiguousarray(x2d.T).astype(bf16)
    woT_eff = np.ascontiguousarray(woT_eff).astype(bf16)

    in_maps = []
    for c in range(NCORES):
        hs = slice(c * F, (c + 1) * F)
        in_maps.append({
            "xT": xT_full,
            "wqT": np.ascontiguousarray((sm_scale * Wq[hs, :]).T).astype(bf16),
            "wkT": np.ascontiguousarray(Wk[hs, :].T).astype(bf16),
            "wvT": np.ascontiguousarray(Wv[hs, :].T).astype(bf16),
            "woT": woT_eff,
            "bqs": np.ascontiguousarray((sm_scale * bq[hs])[:, None]),
            "bks": np.ascontiguousarray(bk[hs][:, None]),
            "bvb": np.ascontiguousarray(
                np.broadcast_to(bv[hs].reshape(HPC, HD)[None], (128, HPC, HD))
            ).astype(bf16),
        })
    return in_maps


def _run_pass(x2d, Wq, bq, Wk, bk, Wv, bv, woT_eff):
    global LAST
    nc = _get_nc()
    in_maps = _make_in_maps(x2d, Wq, bq, Wk, bk, Wv, bv, woT_eff)
    res = run_bass_kernel_spmd(nc, in_maps, list(range(NCORES)), trace=TRACE)
    LAST = res
    return np.concatenate([res.results[c]["out"] for c in range(NCORES)], axis=0)


def kernel(x, Wq, bq, Wk, bk, Wv, bv, Wo, bo,
           variance_reduction_weight, length_adaptive_weight, perms):
    x = np.asarray(x, dtype=np.float32)
    Wq, bq = np.asarray(Wq, np.float32), np.asarray(bq, np.float32)
    Wk, bk = np.asarray(Wk, np.float32), np.asarray(bk, np.float32)
    Wv, bv = np.asarray(Wv, np.float32), np.asarray(bv, np.float32)
    Wo, bo = np.asarray(Wo, np.float32), np.asarray(bo, np.float32)
    perms = np.asarray(perms)
    b, s, d = x.shape

    law = float(np.asarray(length_adaptive_weight).reshape(-1)[0])
    vrw = float(np.asarray(variance_reduction_weight).reshape(-1)[0])
    w = np.float32(min(max(law * np.log(s) / s, 0.01), 1.0))
    x2d = x.reshape(R, D)

    is_perm = all(
        np.array_equal(np.sort(np.asarray(perms[i])), np.arange(s))
        for i in range(perms.shape[0])
    )

    if is_perm:
        c = (1.0 - w) + w * vrw
        outp = _run_pass(x2d, Wq, bq, Wk, bk, Wv, bv, (c * Wo).T)
        outp = outp + (c * bo)[None, :]
        return outp.reshape(b, s, d).astype(np.float32)

    acc = _run_pass(x2d, Wq, bq, Wk, bk, Wv, bv, ((1.0 - w) * Wo).T)
    pscale = (w * vrw) / np.float32(perms.shape[0])
    for i in range(perms.shape[0]):
        perm = np.asarray(perms[i]).astype(np.int64)
        xp = x[:, perm, :].reshape(R, D)
        op = _run_pass(xp, Wq, bq, Wk, bk, Wv, bv, (pscale * Wo).T)
        op3 = op.reshape(b, s, d)
        inv = np.argsort(perm)
        acc += op3[:, inv, :].reshape(R, D)
    acc = acc + (((1.0 - w) + w * vrw) * bo)[None, :]
    return acc.reshape(b, s, d).astype(np.float32)
